# revision 4
# baseline (speedup 1.0000x reference)
"""Two-layer GAT on 8 Trainium2 NeuronCores (Bass/Tile).

Strategy (edge-cut, dst-sharded):
  - Core c owns destination nodes [c*6250, (c+1)*6250).
  - Phase A (replicated): every core computes the full layer-1 feature table
    T1[n] = [h1(256) | a_src1(8)] = x @ [W1 | W1@As | W1@Ad] (fp16), plus
    a_dst1[n], written to local HBM.  No collectives.
  - Phase B: per-core edge aggregation for its own dst nodes.  Edges sorted
    into (<=128-dst-node) tiles; per 128-edge chunk, gather T1[src] rows via
    dma_gather, compute per-edge ex = exp(leaky(a_src+a_dst)), and
    scatter-add via a one-hot (selection-matrix) matmul into PSUM.
    Softmax normalization happens per dst tile at finalize, followed by
    bias+elu and the layer-2 input transform h2 = h @ W2, producing the
    layer-2 table slice T2[own] = [h2(64) | a_src2(1)].
  - Phase C: AllGather of T2 slices across the 8 cores.
  - Phase D: same aggregation machinery for layer 2 (single head), writing
    the final output rows.

dma_gather uses int16 indices, so the gather tables are split in two halves
(lo/hi); each chunk's edges come from a single half (host-side grouping).
"""

import numpy as np

import concourse.bass as bass
import concourse.bacc as bacc
import concourse.tile as tile
import concourse.mybir as mybir
from concourse.bass_utils import run_bass_kernel_spmd
from concourse.masks import make_identity

F32 = mybir.dt.float32
F16 = mybir.dt.float16
I8 = mybir.dt.int8
I16 = mybir.dt.int16
I32 = mybir.dt.int32
A = mybir.AluOpType
AF = mybir.ActivationFunctionType

# -------- problem constants (hardcoded per the task contract) --------
N, E, IN, HID, OUT, H = 50000, 800000, 128, 32, 64, 8
C1 = H * HID  # 256
NCORES = 8
NPC = N // NCORES  # 6250 dst nodes per core
NTILE_A = 391  # ceil(50000/128)
NPAD = NTILE_A * 128  # 50048
T1_LO_TILES = 196
T1_LO = T1_LO_TILES * 128  # 25088 row split of the T1 tables (int16 range)
T1_HI = NPAD - T1_LO  # 24960
T1_W = 384  # T1 row width (768B, multiple of 256B): h(256)+asrc(8)+pad
T2_SLICE = 6272  # per-core T2 rows (6250 + dump/pad)
T2_DUMP = 6260
T2_FULL = T2_SLICE * NCORES  # 50176
T2_LO = 25088  # = 4*6272, row split of T2_full; node n < 25000 <=> row < 25088
CHL = 9  # chunks per (tile, table-half)
CH = 2 * CHL  # chunks per gather call (two tiles' worth of one half)
CALLW = CH * 128  # 2304 gather rows per call
EPS = 1e-16
NEG = 0.2


# ---------------------------------------------------------------------------
# host-side preprocessing
# ---------------------------------------------------------------------------

def _prep_weights(W1, as1, ad1, b1, W2, as2, ad2, b2):
    As = np.zeros((C1, H), np.float32)
    Ad = np.zeros((C1, H), np.float32)
    for h in range(H):
        As[h * HID:(h + 1) * HID, h] = as1[h]
        Ad[h * HID:(h + 1) * HID, h] = ad1[h]
    W1ext = np.concatenate([W1, W1 @ As, W1 @ Ad], axis=1)  # [128, 272]
    return {
        "W1ext": W1ext.astype(np.float16),
        "W2f": W2.astype(np.float16),  # [256, 64]
        "b1rep": np.tile(b1[None, :], (128, 1)).astype(np.float16),
        "b2rep": np.tile(b2[None, :], (128, 1)).astype(np.float32),
        "as2rep": np.tile(as2[0][None, :], (128, 1)).astype(np.float32),
        "ad2rep": np.tile(ad2[0][None, :], (128, 1)).astype(np.float32),
    }


def _greedy_tiles(deg_lo1, deg_hi1, deg_lo2, deg_hi2):
    """Pack the core's 6250 nodes into tiles of width<=128 with each of the
    four per-half edge counts <= CHL*128."""
    cap = CHL * 128
    tiles = []  # (n0, n1) local node ranges
    i, n = 0, len(deg_lo1)
    while i < n:
        l1 = h1 = l2 = h2 = 0
        j = i
        while j < n and j - i < 128:
            nl1, nh1 = l1 + deg_lo1[j], h1 + deg_hi1[j]
            nl2, nh2 = l2 + deg_lo2[j], h2 + deg_hi2[j]
            if nl1 > cap or nh1 > cap or nl2 > cap or nh2 > cap:
                break
            l1, h1, l2, h2 = nl1, nh1, nl2, nh2
            j += 1
        assert j > i, "single node exceeds chunk caps"
        tiles.append((i, j))
        i = j
    return tiles


def _pack_calls(rows_per_tile_half, dloc_per_tile_half, T):
    """Build gather-call arrays.

    rows_per_tile_half[(t, half)] = int array of gather row indices (within
    that half's table) for the tile's edges of that half; dloc likewise the
    dst-local slot.  Call g = (pair pr, half h) covers tiles 2pr (chunks
    0..CHL-1) and 2pr+1 (chunks CHL..CH-1).

    Returns idx16 [128, T*CH*8], dloc8 [128, T*CH] int8,
    dlocT [128, T*CH*128] int8 (partition-replicated).
    """
    ncalls = T  # T/2 pairs * 2 halves
    idx16 = np.zeros((128, ncalls * CH * 8), np.int16)
    dloc8 = np.full((128, ncalls * CH), -1, np.int8)
    dlocT_flat = np.full((ncalls * CH * 128,), -1, np.int8)
    for pr in range(T // 2):
        for h in (0, 1):
            g = 2 * pr + h
            rows = np.zeros((CALLW,), np.int64)
            dl = np.full((CALLW,), -1, np.int64)
            for k, t in enumerate((2 * pr, 2 * pr + 1)):
                r = rows_per_tile_half.get((t, h))
                if r is None:
                    continue
                d = dloc_per_tile_half[(t, h)]
                off = k * CHL * 128
                rows[off:off + len(r)] = r
                dl[off:off + len(r)] = d
            # idx16 layout: position i -> [i%16, g*CH*8 + i//16]
            blk = rows.reshape(CH * 8, 16).T.astype(np.int16)  # [16, CH*8]
            idx16[:, g * CH * 8:(g + 1) * CH * 8] = np.tile(blk, (8, 1))
            # dloc8: position i=j*128+p -> [p, g*CH + j]
            dloc8[:, g * CH:(g + 1) * CH] = dl.reshape(CH, 128).T.astype(np.int8)
            dlocT_flat[g * CALLW:(g + 1) * CALLW] = dl.astype(np.int8)
    dlocT = np.tile(dlocT_flat[None, :], (128, 1))
    return idx16, dloc8, dlocT


def _prep_core(c, src, dst, T_target=None):
    """Per-core host arrays.  src/dst are the full (with self-loops) edge
    arrays, int64."""
    base = c * NPC
    own = (dst >= base) & (dst < base + NPC)
    s = src[own].astype(np.int64)
    d = (dst[own] - base).astype(np.int64)
    order = np.argsort(d, kind="stable")
    s, d = s[order], d[order]
    ptr = np.zeros(NPC + 1, np.int64)
    np.cumsum(np.bincount(d, minlength=NPC), out=ptr[1:])

    lo1m = s < T1_LO
    lo2m = s < 25000
    deg_lo1 = np.bincount(d, weights=lo1m, minlength=NPC).astype(np.int64)
    deg_hi1 = np.bincount(d, weights=~lo1m, minlength=NPC).astype(np.int64)
    deg_lo2 = np.bincount(d, weights=lo2m, minlength=NPC).astype(np.int64)
    deg_hi2 = np.bincount(d, weights=~lo2m, minlength=NPC).astype(np.int64)
    tiles = _greedy_tiles(deg_lo1, deg_hi1, deg_lo2, deg_hi2)
    Treal = len(tiles)

    owner = s // NPC
    t2row = owner * T2_SLICE + (s - owner * NPC)

    rows1, dloc1 = {}, {}
    rows2, dloc2 = {}, {}
    for t, (n0, n1) in enumerate(tiles):
        e0, e1 = ptr[n0], ptr[n1]
        es, ed = s[e0:e1], d[e0:e1]
        dl = ed - n0
        m1 = es < T1_LO
        rows1[(t, 0)] = es[m1]
        dloc1[(t, 0)] = dl[m1]
        rows1[(t, 1)] = es[~m1] - T1_LO
        dloc1[(t, 1)] = dl[~m1]
        m2 = es < 25000
        rows2[(t, 0)] = t2row[e0:e1][m2]
        dloc2[(t, 0)] = dl[m2]
        rows2[(t, 1)] = t2row[e0:e1][~m2] - T2_LO
        dloc2[(t, 1)] = dl[~m2]

    return {
        "Treal": Treal,
        "tiles": tiles,
        "rows1": rows1, "dloc1": dloc1,
        "rows2": rows2, "dloc2": dloc2,
    }


def _finish_core(pc, c, T):
    """Pad to T tiles and build the final input arrays for core c."""
    base = c * NPC
    tiles = list(pc["tiles"]) + [(0, 0)] * (T - pc["Treal"])
    idx1, dloc1, dlocT1 = _pack_calls(pc["rows1"], pc["dloc1"], T)
    idx2, dloc2, dlocT2 = _pack_calls(pc["rows2"], pc["dloc2"], T)

    adrow = np.zeros((128, T), np.int32)
    t2w = np.full((128, T), T2_DUMP, np.int32)
    outw = np.full((128, T), T2_DUMP, np.int32)
    for t, (n0, n1) in enumerate(tiles):
        w = n1 - n0
        p = np.arange(128)
        adrow[:, t] = base + n0 + np.minimum(p, max(w - 1, 0))
        if w > 0:
            t2w[:w, t] = n0 + p[:w]
            outw[:w, t] = n0 + p[:w]
    return {
        "g1idx": idx1, "dloc1": dloc1, "dlocT1": dlocT1,
        "g2idx": idx2, "dloc2": dloc2, "dlocT2": dlocT2,
        "adrow": adrow, "t2w": t2w, "outw": outw,
    }


def host_prep(inputs):
    x = np.asarray(inputs["x"], np.float32)
    ei = np.asarray(inputs["edge_index"]).astype(np.int64)
    wd = _prep_weights(
        np.asarray(inputs["W1"], np.float32),
        np.asarray(inputs["att_src1"], np.float32),
        np.asarray(inputs["att_dst1"], np.float32),
        np.asarray(inputs["b1"], np.float32),
        np.asarray(inputs["W2"], np.float32),
        np.asarray(inputs["att_src2"], np.float32),
        np.asarray(inputs["att_dst2"], np.float32),
        np.asarray(inputs["b2"], np.float32),
    )
    loops = np.arange(N, dtype=np.int64)
    src = np.concatenate([ei[0], loops])
    dst = np.concatenate([ei[1], loops])

    xT = np.zeros((IN, NPAD), np.float16)
    xT[:, :N] = x.T.astype(np.float16)

    cores = [_prep_core(c, src, dst) for c in range(NCORES)]
    T = max(pc["Treal"] for pc in cores)
    if T % 2:
        T += 1
    per_core = [_finish_core(pc, c, T) for c, pc in enumerate(cores)]

    common = dict(wd)
    common["xT"] = xT
    return T, common, per_core


# ---------------------------------------------------------------------------
# device program
# ---------------------------------------------------------------------------

def build_nc(T, num_devices=NCORES, with_collective=True):
    nc = bacc.Bacc("TRN2", target_bir_lowering=False, debug=False,
                   num_devices=num_devices)
    dt = nc.dram_tensor
    # inputs (common)
    xT = dt("xT", [IN, NPAD], F16, kind="ExternalInput").ap()
    W1ext = dt("W1ext", [128, 272], F16, kind="ExternalInput").ap()
    W2f = dt("W2f", [256, 64], F16, kind="ExternalInput").ap()
    b1rep = dt("b1rep", [128, 256], F16, kind="ExternalInput").ap()
    b2rep = dt("b2rep", [128, 64], F32, kind="ExternalInput").ap()
    as2rep = dt("as2rep", [128, 64], F32, kind="ExternalInput").ap()
    ad2rep = dt("ad2rep", [128, 64], F32, kind="ExternalInput").ap()
    # inputs (per core)
    g1idx = dt("g1idx", [128, T * CH * 8], I16, kind="ExternalInput").ap()
    g2idx = dt("g2idx", [128, T * CH * 8], I16, kind="ExternalInput").ap()
    dloc1 = dt("dloc1", [128, T * CH], I8, kind="ExternalInput").ap()
    dloc2 = dt("dloc2", [128, T * CH], I8, kind="ExternalInput").ap()
    dlocT1 = dt("dlocT1", [128, T * CH * 128], I8, kind="ExternalInput").ap()
    dlocT2 = dt("dlocT2", [128, T * CH * 128], I8, kind="ExternalInput").ap()
    adrow = dt("adrow", [128, T], I32, kind="ExternalInput").ap()
    t2w = dt("t2w", [128, T], I32, kind="ExternalInput").ap()
    outw = dt("outw", [128, T], I32, kind="ExternalInput").ap()
    # internal tables
    T1lo = dt("T1lo", [T1_LO, T1_W], F16, kind="Internal").ap()
    T1hi = dt("T1hi", [T1_HI, T1_W], F16, kind="Internal").ap()
    adst1 = dt("adst1", [NPAD, 8], F16, kind="Internal").ap()
    t2slice = dt("t2slice", [T2_SLICE, 128], F16, kind="Internal").ap()
    if with_collective:
        t2full = dt("t2full", [T2_FULL, 128], F16, kind="Internal",
                    addr_space="Shared").ap()
    else:
        t2full = dt("t2full", [T2_FULL, 128], F16, kind="Internal").ap()
    # output
    outp = dt("out", [T2_SLICE, 64], F32, kind="ExternalOutput").ap()

    with tile.TileContext(nc) as tc:
        with tc.tile_pool(name="consts", bufs=1) as cp:
            W1e_sb = cp.tile([128, 272], F16)
            nc.sync.dma_start(out=W1e_sb[:], in_=W1ext[:])
            W2a_sb = cp.tile([128, 64], F16)
            nc.sync.dma_start(out=W2a_sb[:], in_=W2f[0:128, :])
            W2b_sb = cp.tile([128, 64], F16)
            nc.sync.dma_start(out=W2b_sb[:], in_=W2f[128:256, :])
            b1_sb = cp.tile([128, 256], F16)
            nc.sync.dma_start(out=b1_sb[:], in_=b1rep[:])
            b2_sb = cp.tile([128, 64], F32)
            nc.sync.dma_start(out=b2_sb[:], in_=b2rep[:])
            as2_sb = cp.tile([128, 64], F32)
            nc.sync.dma_start(out=as2_sb[:], in_=as2rep[:])
            ad2_sb = cp.tile([128, 64], F32)
            nc.sync.dma_start(out=ad2_sb[:], in_=ad2rep[:])
            iota_row = cp.tile([128, 128], I8)
            nc.gpsimd.iota(iota_row[:], pattern=[[1, 128]], base=0,
                           channel_multiplier=0,
                           allow_small_or_imprecise_dtypes=True)
            iota_col = cp.tile([128, 1], I8)
            nc.gpsimd.iota(iota_col[:], pattern=[[0, 1]], base=0,
                           channel_multiplier=1,
                           allow_small_or_imprecise_dtypes=True)
            idn = cp.tile([128, 128], F16)
            make_identity(nc, idn[:])
            g1i_sb = cp.tile([128, T * CH * 8], I16)
            nc.sync.dma_start(out=g1i_sb[:], in_=g1idx[:])
            g2i_sb = cp.tile([128, T * CH * 8], I16)
            nc.sync.dma_start(out=g2i_sb[:], in_=g2idx[:])
            dl1_sb = cp.tile([128, T * CH], I8)
            nc.sync.dma_start(out=dl1_sb[:], in_=dloc1[:])
            dl2_sb = cp.tile([128, T * CH], I8)
            nc.sync.dma_start(out=dl2_sb[:], in_=dloc2[:])
            adrow_sb = cp.tile([128, T], I32)
            nc.sync.dma_start(out=adrow_sb[:], in_=adrow[:])
            t2w_sb = cp.tile([128, T], I32)
            nc.sync.dma_start(out=t2w_sb[:], in_=t2w[:])
            outw_sb = cp.tile([128, T], I32)
            nc.sync.dma_start(out=outw_sb[:], in_=outw[:])
            adst2_sb = cp.tile([128, T], F16)  # written phase B, read phase D

            # ---------------- Phase A: T1 build (replicated) ----------------
            with tc.tile_pool(name="pa", bufs=2) as pa, \
                 tc.tile_pool(name="paps", bufs=2, space="PSUM") as paps:
                XB = 2048  # nodes per xT block
                for blk in range((NPAD + XB - 1) // XB):
                    n0 = blk * XB
                    bw = min(XB, NPAD - n0)
                    xb = pa.tile([128, bw], F16, tag="xb")
                    nc.sync.dma_start(out=xb[:], in_=xT[:, n0:n0 + bw])
                    for i in range(bw // 128):
                        t = (n0 + i * 128) // 128  # global tile index
                        ps = paps.tile([128, 272], F32, tag="aps")
                        nc.tensor.matmul(ps[:], lhsT=xb[:, i * 128:(i + 1) * 128],
                                         rhs=W1e_sb[:], start=True, stop=True)
                        t1r = pa.tile([128, 264], F16, tag="t1r")
                        nc.scalar.copy(t1r[:], ps[:, 0:264])
                        ad = pa.tile([128, 8], F16, tag="ad")
                        nc.vector.tensor_copy(ad[:], ps[:, 264:272])
                        if t < T1_LO_TILES:
                            dst_ap = T1lo[t * 128:(t + 1) * 128, 0:264]
                        else:
                            r0 = (t - T1_LO_TILES) * 128
                            dst_ap = T1hi[r0:r0 + 128, 0:264]
                        nc.sync.dma_start(out=dst_ap, in_=t1r[:])
                        nc.scalar.dma_start(out=adst1[t * 128:(t + 1) * 128, :],
                                            in_=ad[:])

            # ---------------- Phase B: layer-1 aggregation ----------------
            _agg_layer(nc, tc, T, layer=1,
                       tbl_lo=T1lo[:], tbl_hi=T1hi[:],
                       gidx_sb=g1i_sb, dloc_sb=dl1_sb, dlocT_in=dlocT1,
                       iota_row=iota_row, iota_col=iota_col, idn=idn,
                       adrow_sb=adrow_sb, adst1=adst1,
                       b1_sb=b1_sb, W2a_sb=W2a_sb, W2b_sb=W2b_sb,
                       as2_sb=as2_sb, ad2_sb=ad2_sb, adst2_sb=adst2_sb,
                       t2w_sb=t2w_sb, t2slice=t2slice,
                       b2_sb=None, outw_sb=None, outp=None)

            # ---------------- Phase C: AllGather T2 ----------------
            if with_collective:
                nc.gpsimd.collective_compute(
                    "AllGather", A.bypass,
                    replica_groups=[list(range(NCORES))],
                    ins=[t2slice[:]], outs=[t2full[:]],
                )
            else:
                # timing-only variant: local copy stands in for the collective
                nc.sync.dma_start(out=t2full[0:T2_SLICE, :], in_=t2slice[:])

            # ---------------- Phase D: layer-2 aggregation ----------------
            _agg_layer(nc, tc, T, layer=2,
                       tbl_lo=t2full[0:T2_LO, :], tbl_hi=t2full[T2_LO:T2_FULL, :],
                       gidx_sb=g2i_sb, dloc_sb=dl2_sb, dlocT_in=dlocT2,
                       iota_row=iota_row, iota_col=iota_col, idn=idn,
                       adrow_sb=None, adst1=None,
                       b1_sb=None, W2a_sb=None, W2b_sb=None,
                       as2_sb=None, ad2_sb=None, adst2_sb=adst2_sb,
                       t2w_sb=None, t2slice=None,
                       b2_sb=b2_sb, outw_sb=outw_sb, outp=outp)

    nc.compile()
    return nc


def _agg_layer(nc, tc, T, layer, tbl_lo, tbl_hi, gidx_sb, dloc_sb, dlocT_in,
               iota_row, iota_col, idn, adrow_sb, adst1, b1_sb, W2a_sb, W2b_sb,
               as2_sb, ad2_sb, adst2_sb, t2w_sb, t2slice, b2_sb, outw_sb, outp):
    L1 = layer == 1
    GW = T1_W if L1 else 128  # gathered row width
    WW = 264 if L1 else 65    # w tile width (values + ex columns)
    NH = 8 if L1 else 1       # heads
    ACC_W = 264 if L1 else 65
    name = f"l{layer}"
    with tc.tile_pool(name=f"pb_{name}", bufs=2) as pb, \
         tc.tile_pool(name=f"pf_{name}", bufs=2) as pf, \
         tc.tile_pool(name=f"ps_acc_{name}", bufs=3, space="PSUM") as ps_acc, \
         tc.tile_pool(name=f"ps_ad_{name}", bufs=2, space="PSUM") as ps_ad, \
         tc.tile_pool(name=f"ps_fin_{name}", bufs=1, space="PSUM") as ps_fin:
        for pr in range(T // 2):
            accs = [ps_acc.tile([128, ACC_W], F32, tag="acc", name="acc_a"),
                    ps_acc.tile([128, ACC_W], F32, tag="acc", name="acc_b")]
            if L1:
                adts = []
                for k in (0, 1):
                    t = 2 * pr + k
                    adt = pb.tile([128, 8], F16, tag=f"adt{k}")
                    nc.gpsimd.indirect_dma_start(
                        out=adt[:], out_offset=None, in_=adst1,
                        in_offset=bass.IndirectOffsetOnAxis(
                            ap=adrow_sb[:, t:t + 1], axis=0))
                    adts.append(adt)
            for hf in (0, 1):
                g = 2 * pr + hf
                gt = pb.tile([128, CH, GW], F16, tag="gt")
                nc.gpsimd.dma_gather(
                    gt[:], tbl_lo if hf == 0 else tbl_hi,
                    gidx_sb[:, g * CH * 8:(g + 1) * CH * 8],
                    CALLW, CALLW, GW, single_packet=False)
                dlT = pb.tile([128, CH, 128], I8, tag="dlT")
                nc.scalar.dma_start(
                    out=dlT[:].rearrange("p j e -> p (j e)"),
                    in_=dlocT_in[:, g * CALLW:(g + 1) * CALLW])
                sel = pb.tile([128, CH, 128], F16, tag="sel")
                nc.vector.tensor_tensor(
                    out=sel[:],
                    in0=dloc_sb[:, g * CH:(g + 1) * CH, None].to_broadcast(
                        [128, CH, 128]),
                    in1=iota_row[:, None, :].to_broadcast([128, CH, 128]),
                    op=A.is_equal)
                selT = pb.tile([128, CH, 128], F16, tag="selT")
                nc.vector.tensor_tensor(
                    out=selT[:],
                    in0=iota_col[:, :, None].to_broadcast([128, CH, 128]),
                    in1=dlT[:],
                    op=A.is_equal)
                # per-edge a_dst via one-hot matmul (segmented broadcast)
                adps = ps_ad.tile([128, CH, NH], F32, tag="adps")
                for j in range(CH):
                    t = 2 * pr + (0 if j < CHL else 1)
                    if L1:
                        rhs = adts[0 if j < CHL else 1][:]
                    else:
                        rhs = adst2_sb[:, t:t + 1]
                    nc.tensor.matmul(adps[:, j, :], lhsT=selT[:, j, :], rhs=rhs,
                                     start=True, stop=True)
                adf = pb.tile([128, CH, NH], F16, tag="adf")
                nc.scalar.copy(adf[:], adps[:])
                et = pb.tile([128, CH, NH], F16, tag="et")
                if L1:
                    asrc_ap = gt[:, :, 256:264]
                else:
                    asrc_ap = gt[:, :, 64:65]
                nc.vector.tensor_tensor(out=et[:], in0=asrc_ap, in1=adf[:],
                                        op=A.add)
                lk = pb.tile([128, CH, NH], F16, tag="lk")
                nc.vector.scalar_tensor_tensor(out=lk[:], in0=et[:], scalar=NEG,
                                               in1=et[:], op0=A.mult, op1=A.max)
                w = pb.tile([128, CH, WW], F16, tag="w")
                nc.scalar.activation(w[:, :, WW - NH:WW], lk[:], AF.Exp)
                if L1:
                    nc.vector.tensor_tensor(
                        out=w[:, :, 0:256].rearrange("p j (h c) -> p j h c", h=8),
                        in0=gt[:, :, 0:256].rearrange("p j (h c) -> p j h c", h=8),
                        in1=w[:, :, 256:264][:, :, :, None].to_broadcast(
                            [128, CH, 8, 32]),
                        op=A.mult)
                else:
                    nc.vector.tensor_tensor(
                        out=w[:, :, 0:64],
                        in0=gt[:, :, 0:64],
                        in1=w[:, :, 64:65].to_broadcast([128, CH, 64]),
                        op=A.mult)
                for j in range(CH):
                    acc = accs[0 if j < CHL else 1]
                    st = (hf == 0) and (j % CHL == 0)
                    sp = (hf == 1) and (j % CHL == CHL - 1)
                    nc.tensor.matmul(acc[:], lhsT=sel[:, j, :], rhs=w[:, j, :],
                                     start=st, stop=sp)
            # finalize both tiles of the pair
            for k in (0, 1):
                t = 2 * pr + k
                acc = accs[k]
                if L1:
                    _fin_l1(nc, tc, t, acc, pf, ps_fin, idn, b1_sb, W2a_sb,
                            W2b_sb, as2_sb, ad2_sb, adst2_sb, t2w_sb, t2slice)
                else:
                    _fin_l2(nc, t, acc, pf, b2_sb, outw_sb, outp)


def _fin_l1(nc, tc, t, acc, pf, ps_fin, idn, b1_sb, W2a_sb, W2b_sb, as2_sb,
            ad2_sb, adst2_sb, t2w_sb, t2slice):
    deps = pf.tile([128, 8], F32, tag="deps")
    nc.vector.tensor_scalar_add(deps[:], acc[:, 256:264], EPS)
    rec = pf.tile([128, 8], F32, tag="rec")
    nc.vector.reciprocal(rec[:], deps[:])
    h1b = pf.tile([128, 256], F16, tag="h1b")
    nc.vector.tensor_tensor(
        out=h1b[:].rearrange("p (h c) -> p h c", h=8),
        in0=acc[:, 0:256].rearrange("p (h c) -> p h c", h=8),
        in1=rec[:, :, None].to_broadcast([128, 8, 32]),
        op=A.mult)
    nc.vector.tensor_tensor(out=h1b[:], in0=h1b[:], in1=b1_sb[:], op=A.add)
    # elu = max(x,0) + exp(min(x,0)) - 1
    mn = pf.tile([128, 256], F16, tag="mn")
    nc.vector.tensor_scalar_min(mn[:], h1b[:], 0.0)
    em = pf.tile([128, 256], F16, tag="em")
    nc.scalar.activation(em[:], mn[:], AF.Exp)
    ho = pf.tile([128, 256], F16, tag="ho")
    nc.vector.tensor_scalar_max(ho[:], h1b[:], 0.0)
    nc.vector.tensor_tensor(out=ho[:], in0=ho[:], in1=em[:], op=A.add)
    nc.vector.tensor_scalar_add(ho[:], ho[:], -1.0)
    # h2 = ho @ W2 via two transposed matmuls
    h2ps = ps_fin.tile([128, 64], F32, tag="h2ps")
    for half in (0, 1):
        tp = ps_fin.tile([128, 128], F16, tag="tp")
        nc.tensor.transpose(out=tp[:], in_=ho[:, half * 128:(half + 1) * 128],
                            identity=idn[:])
        hoT = pf.tile([128, 128], F16, tag="hoT")
        nc.scalar.copy(hoT[:], tp[:])
        nc.tensor.matmul(h2ps[:], lhsT=hoT[:],
                         rhs=(W2a_sb if half == 0 else W2b_sb)[:],
                         start=half == 0, stop=half == 1)
    t2r = pf.tile([128, 128], F16, tag="t2r")
    nc.scalar.copy(t2r[:, 0:64], h2ps[:])
    tmp = pf.tile([128, 64], F32, tag="tmp")
    nc.vector.tensor_tensor(out=tmp[:], in0=h2ps[:], in1=as2_sb[:], op=A.mult)
    a2s = pf.tile([128, 1], F32, tag="a2s")
    nc.vector.reduce_sum(a2s[:], tmp[:], axis=mybir.AxisListType.X)
    nc.vector.tensor_copy(t2r[:, 64:65], a2s[:])
    nc.vector.tensor_tensor(out=tmp[:], in0=h2ps[:], in1=ad2_sb[:], op=A.mult)
    a2d = pf.tile([128, 1], F32, tag="a2d")
    nc.vector.reduce_sum(a2d[:], tmp[:], axis=mybir.AxisListType.X)
    nc.vector.tensor_copy(adst2_sb[:, t:t + 1], a2d[:])
    nc.gpsimd.indirect_dma_start(
        out=t2slice, out_offset=bass.IndirectOffsetOnAxis(
            ap=t2w_sb[:, t:t + 1], axis=0),
        in_=t2r[:], in_offset=None)


def _fin_l2(nc, t, acc, pf, b2_sb, outw_sb, outp):
    dep = pf.tile([128, 1], F32, tag="dep2")
    nc.vector.tensor_scalar_add(dep[:], acc[:, 64:65], EPS)
    rec = pf.tile([128, 1], F32, tag="rec2")
    nc.vector.reciprocal(rec[:], dep[:])
    ot = pf.tile([128, 64], F32, tag="ot")
    nc.vector.tensor_scalar_mul(ot[:], acc[:, 0:64], rec[:, 0:1])
    nc.vector.tensor_tensor(out=ot[:], in0=ot[:], in1=b2_sb[:], op=A.add)
    nc.gpsimd.indirect_dma_start(
        out=outp, out_offset=bass.IndirectOffsetOnAxis(
            ap=outw_sb[:, t:t + 1], axis=0),
        in_=ot[:], in_offset=None)


# ---------------------------------------------------------------------------
# entry point
# ---------------------------------------------------------------------------

def make_in_maps(T, common, per_core):
    in_maps = []
    for c in range(NCORES):
        m = {
            "xT": common["xT"], "W1ext": common["W1ext"], "W2f": common["W2f"],
            "b1rep": common["b1rep"], "b2rep": common["b2rep"],
            "as2rep": common["as2rep"], "ad2rep": common["ad2rep"],
        }
        m.update(per_core[c])
        in_maps.append(m)
    return in_maps


def kernel(**inputs):
    T, common, per_core = host_prep(inputs)
    nc = build_nc(T)
    in_maps = make_in_maps(T, common, per_core)
    res = run_bass_kernel_spmd(nc, in_maps, core_ids=list(range(NCORES)))
    out = np.concatenate([res.results[c]["out"][:NPC] for c in range(NCORES)],
                         axis=0)
    return out.astype(np.float32)


# revision 10
# speedup vs baseline: 1.2659x; 1.2659x over previous
"""Two-layer GAT on 8 Trainium2 NeuronCores (Bass/Tile).

Strategy (edge-cut, dst-sharded):
  - Core c owns destination nodes [c*6250, (c+1)*6250).
  - Phase A (replicated): every core computes the full layer-1 feature table
    T1[n] = [h1(256) | a_src1(8)] = x @ [W1 | W1@As | W1@Ad] (fp16), plus
    a_dst1[n], written to local HBM.  No collectives.
  - Phase B: per-core edge aggregation for its own dst nodes.  Edges sorted
    into (<=128-dst-node) tiles; per 128-edge chunk, gather T1[src] rows via
    dma_gather, compute per-edge ex = exp(leaky(a_src+a_dst)), and
    scatter-add via a one-hot (selection-matrix) matmul into PSUM.
    Softmax normalization happens per dst tile at finalize, followed by
    bias+elu and the layer-2 input transform h2 = h @ W2, producing the
    layer-2 table slice T2[own] = [h2(64) | a_src2(1)].
  - Phase C: AllGather of T2 slices across the 8 cores.
  - Phase D: same aggregation machinery for layer 2 (single head), writing
    the final output rows.

dma_gather uses int16 indices, so the gather tables are addressed in two
halves (lo/hi row slices); each 128-edge chunk's edges come from a single
half (host-side grouping).  Selection matrices are built on the DVE in
layouts where every operand streams at unit stride (2x perf mode).
"""

import numpy as np

import concourse.bass as bass
import concourse.bacc as bacc
import concourse.tile as tile
import concourse.mybir as mybir
from concourse.bass_utils import run_bass_kernel_spmd
from concourse.masks import make_identity

F32 = mybir.dt.float32
F16 = mybir.dt.float16
I16 = mybir.dt.int16
I32 = mybir.dt.int32
A = mybir.AluOpType
AF = mybir.ActivationFunctionType

# -------- problem constants (hardcoded per the task contract) --------
N, E, IN, HID, OUT, H = 50000, 800000, 128, 32, 64, 8
C1 = H * HID  # 256
NCORES = 8
NPC = N // NCORES  # 6250 dst nodes per core
NTILE_A = 391  # ceil(50000/128)
NPAD = NTILE_A * 128  # 50048
T1_LO = 25088  # row split of T1 (int16 index range)
T1_W = 384  # T1 row width (768B, multiple of 256B): h(256)+asrc(8)+pad
T2_SLICE = 6272  # per-core T2 rows (6250 + dump/pad)
T2_DUMP = 6260
T2_FULL = T2_SLICE * NCORES  # 50176
T2_LO = 25088  # row split of T2_full; node n < 25000 <=> row < 25088
CHL = 9  # chunks per (tile, table-half)
CH = 2 * CHL  # chunks per gather call (two tiles' worth of one half)
CALLW = CH * 128  # gather rows per call
EPS = 1e-16
NEG = 0.2


# ---------------------------------------------------------------------------
# host-side preprocessing
# ---------------------------------------------------------------------------

def _prep_weights(W1, as1, ad1, b1, W2, as2, ad2, b2):
    As = np.zeros((C1, H), np.float32)
    Ad = np.zeros((C1, H), np.float32)
    for h in range(H):
        As[h * HID:(h + 1) * HID, h] = as1[h]
        Ad[h * HID:(h + 1) * HID, h] = ad1[h]
    W1ext = np.concatenate([W1, W1 @ As, W1 @ Ad], axis=1)  # [128, 272]
    iotarep = np.tile(np.arange(128, dtype=np.float16)[:, None],
                      (1, CH)).reshape(1, 128 * CH)
    return {
        "W1ext": W1ext.astype(np.float16),
        "W2f": W2.astype(np.float16),  # [256, 64]
        "b1rep": np.tile(b1[None, :], (128, 1)).astype(np.float16),
        "b2rep": np.tile(b2[None, :], (128, 1)).astype(np.float32),
        "as2rep": np.tile(as2[0][None, :], (128, 1)).astype(np.float32),
        "ad2rep": np.tile(ad2[0][None, :], (128, 1)).astype(np.float32),
        # iotarep[p, d*CH + j] = d
        "iotarep": np.tile(iotarep, (128, 1)),
        # pcol[p, e] = p
        "pcol": np.tile(np.arange(128, dtype=np.float16)[:, None], (1, 128)),
    }


def _greedy_tiles(deg_lo1, deg_hi1, deg_lo2, deg_hi2):
    """Pack the core's 6250 nodes into tiles of width<=128 with each of the
    four per-half edge counts <= CHL*128."""
    cap = CHL * 128
    tiles = []  # (n0, n1) local node ranges
    i, n = 0, len(deg_lo1)
    while i < n:
        l1 = h1 = l2 = h2 = 0
        j = i
        while j < n and j - i < 128:
            nl1, nh1 = l1 + deg_lo1[j], h1 + deg_hi1[j]
            nl2, nh2 = l2 + deg_lo2[j], h2 + deg_hi2[j]
            if nl1 > cap or nh1 > cap or nl2 > cap or nh2 > cap:
                break
            l1, h1, l2, h2 = nl1, nh1, nl2, nh2
            j += 1
        assert j > i, "single node exceeds chunk caps"
        tiles.append((i, j))
        i = j
    return tiles


def _pack_calls(rows_per_tile_half, dloc_per_tile_half, T):
    """Build gather-call arrays.

    Call g = (pair pr, half h) covers tiles 2pr (chunks 0..CHL-1) and 2pr+1
    (chunks CHL..CH-1).  Edge position i in a call lands on SBUF partition
    i%128, chunk i//128 (dma_gather layout), with the int16 index read from
    idx16[i%16, g*CH*8 + i//16].

    Returns idx16 [128, T*CH*8] i16, dloc [128, T*CH] f16,
    dlocT [128, T*CH*128] f16 (partition-replicated transpose).
    """
    ncalls = T
    idx16 = np.zeros((128, ncalls * CH * 8), np.int16)
    dloc = np.full((128, ncalls * CH), -1, np.float16)
    dlocT_flat = np.full((ncalls * CH * 128,), -1, np.float16)
    for pr in range(T // 2):
        for h in (0, 1):
            g = 2 * pr + h
            rows = np.zeros((CALLW,), np.int64)
            dl = np.full((CALLW,), -1, np.int64)
            for k, t in enumerate((2 * pr, 2 * pr + 1)):
                r = rows_per_tile_half.get((t, h))
                if r is None:
                    continue
                d = dloc_per_tile_half[(t, h)]
                off = k * CHL * 128
                rows[off:off + len(r)] = r
                dl[off:off + len(r)] = d
            blk = rows.reshape(CH * 8, 16).T.astype(np.int16)  # [16, CH*8]
            idx16[:, g * CH * 8:(g + 1) * CH * 8] = np.tile(blk, (8, 1))
            dloc[:, g * CH:(g + 1) * CH] = dl.reshape(CH, 128).T.astype(np.float16)
            dlocT_flat[g * CALLW:(g + 1) * CALLW] = dl.astype(np.float16)
    dlocT = np.tile(dlocT_flat[None, :], (128, 1))
    return idx16, dloc, dlocT


def _prep_core(c, src, dst):
    base = c * NPC
    own = (dst >= base) & (dst < base + NPC)
    s = src[own].astype(np.int64)
    d = (dst[own] - base).astype(np.int64)
    order = np.argsort(d, kind="stable")
    s, d = s[order], d[order]
    ptr = np.zeros(NPC + 1, np.int64)
    np.cumsum(np.bincount(d, minlength=NPC), out=ptr[1:])

    lo1m = s < T1_LO
    lo2m = s < 25000
    deg_lo1 = np.bincount(d, weights=lo1m, minlength=NPC).astype(np.int64)
    deg_hi1 = np.bincount(d, weights=~lo1m, minlength=NPC).astype(np.int64)
    deg_lo2 = np.bincount(d, weights=lo2m, minlength=NPC).astype(np.int64)
    deg_hi2 = np.bincount(d, weights=~lo2m, minlength=NPC).astype(np.int64)
    tiles = _greedy_tiles(deg_lo1, deg_hi1, deg_lo2, deg_hi2)

    owner = s // NPC
    t2row = owner * T2_SLICE + (s - owner * NPC)

    rows1, dloc1 = {}, {}
    rows2, dloc2 = {}, {}
    for t, (n0, n1) in enumerate(tiles):
        e0, e1 = ptr[n0], ptr[n1]
        es, ed = s[e0:e1], d[e0:e1]
        dl = ed - n0
        m1 = es < T1_LO
        rows1[(t, 0)] = es[m1]
        dloc1[(t, 0)] = dl[m1]
        rows1[(t, 1)] = es[~m1] - T1_LO
        dloc1[(t, 1)] = dl[~m1]
        m2 = es < 25000
        rows2[(t, 0)] = t2row[e0:e1][m2]
        dloc2[(t, 0)] = dl[m2]
        rows2[(t, 1)] = t2row[e0:e1][~m2] - T2_LO
        dloc2[(t, 1)] = dl[~m2]

    return {
        "Treal": len(tiles), "tiles": tiles,
        "rows1": rows1, "dloc1": dloc1,
        "rows2": rows2, "dloc2": dloc2,
    }


def _finish_core(pc, c, T):
    tiles = list(pc["tiles"]) + [(0, 0)] * (T - pc["Treal"])
    idx1, dloc1, dlocT1 = _pack_calls(pc["rows1"], pc["dloc1"], T)
    idx2, dloc2, dlocT2 = _pack_calls(pc["rows2"], pc["dloc2"], T)

    base = c * NPC
    adrow = np.zeros((128, T), np.int32)
    t2w = np.full((128, T), T2_DUMP, np.int32)
    outw = np.full((128, T), T2_DUMP, np.int32)
    p = np.arange(128)
    for t, (n0, n1) in enumerate(tiles):
        w = n1 - n0
        adrow[:, t] = base + n0 + np.minimum(p, max(w - 1, 0))
        if w > 0:
            t2w[:w, t] = n0 + p[:w]
            outw[:w, t] = n0 + p[:w]
    return {
        "g1idx": idx1, "dloc1": dloc1, "dlocT1": dlocT1,
        "g2idx": idx2, "dloc2": dloc2, "dlocT2": dlocT2,
        "adrow": adrow, "t2w": t2w, "outw": outw,
    }


def host_prep(inputs):
    ei = np.asarray(inputs["edge_index"]).astype(np.int64)
    wd = _prep_weights(
        np.asarray(inputs["W1"], np.float32),
        np.asarray(inputs["att_src1"], np.float32),
        np.asarray(inputs["att_dst1"], np.float32),
        np.asarray(inputs["b1"], np.float32),
        np.asarray(inputs["W2"], np.float32),
        np.asarray(inputs["att_src2"], np.float32),
        np.asarray(inputs["att_dst2"], np.float32),
        np.asarray(inputs["b2"], np.float32),
    )
    loops = np.arange(N, dtype=np.int64)
    src = np.concatenate([ei[0], loops])
    dst = np.concatenate([ei[1], loops])

    xT = np.zeros((IN, NPAD), np.float16)
    xT[:, :N] = np.asarray(inputs["x"], np.float32).T.astype(np.float16)

    cores = [_prep_core(c, src, dst) for c in range(NCORES)]
    T = max(pc["Treal"] for pc in cores)
    if T % 2:
        T += 1
    per_core = [_finish_core(pc, c, T) for c, pc in enumerate(cores)]

    common = dict(wd)
    common["xT"] = xT
    return T, common, per_core


# ---------------------------------------------------------------------------
# device program
# ---------------------------------------------------------------------------

def build_nc(T, num_devices=NCORES, with_collective=True, phases="ABCD"):
    nc = bacc.Bacc("TRN2", target_bir_lowering=False, debug=False,
                   num_devices=num_devices)
    dt = nc.dram_tensor
    xT = dt("xT", [IN, NPAD], F16, kind="ExternalInput").ap()
    W1ext = dt("W1ext", [128, 272], F16, kind="ExternalInput").ap()
    W2f = dt("W2f", [256, 64], F16, kind="ExternalInput").ap()
    b1rep = dt("b1rep", [128, 256], F16, kind="ExternalInput").ap()
    b2rep = dt("b2rep", [128, 64], F32, kind="ExternalInput").ap()
    as2rep = dt("as2rep", [128, 64], F32, kind="ExternalInput").ap()
    ad2rep = dt("ad2rep", [128, 64], F32, kind="ExternalInput").ap()
    iotarep = dt("iotarep", [128, 128 * CH], F16, kind="ExternalInput").ap()
    pcol = dt("pcol", [128, 128], F16, kind="ExternalInput").ap()
    g1idx = dt("g1idx", [128, T * CH * 8], I16, kind="ExternalInput").ap()
    g2idx = dt("g2idx", [128, T * CH * 8], I16, kind="ExternalInput").ap()
    dloc1 = dt("dloc1", [128, T * CH], F16, kind="ExternalInput").ap()
    dloc2 = dt("dloc2", [128, T * CH], F16, kind="ExternalInput").ap()
    dlocT1 = dt("dlocT1", [128, T * CH * 128], F16, kind="ExternalInput").ap()
    dlocT2 = dt("dlocT2", [128, T * CH * 128], F16, kind="ExternalInput").ap()
    adrow = dt("adrow", [128, T], I32, kind="ExternalInput").ap()
    t2w = dt("t2w", [128, T], I32, kind="ExternalInput").ap()
    outw = dt("outw", [128, T], I32, kind="ExternalInput").ap()
    T1 = dt("T1", [NPAD, T1_W], F16, kind="Internal").ap()
    adst1 = dt("adst1", [NPAD, 8], F16, kind="Internal").ap()
    t2slice = dt("t2slice", [T2_SLICE, 128], F16, kind="Internal").ap()
    t2full = dt("t2full", [T2_FULL, 128], F16, kind="Internal",
                addr_space="Shared" if with_collective else "Local").ap()
    outp = dt("out", [T2_SLICE, 64], F32, kind="ExternalOutput").ap()

    with tile.TileContext(nc) as tc:
        with tc.tile_pool(name="consts", bufs=1) as cp:
            W1e_sb = cp.tile([128, 272], F16)
            nc.sync.dma_start(out=W1e_sb[:], in_=W1ext[:])
            W2a_sb = cp.tile([128, 64], F16)
            nc.sync.dma_start(out=W2a_sb[:], in_=W2f[0:128, :])
            W2b_sb = cp.tile([128, 64], F16)
            nc.sync.dma_start(out=W2b_sb[:], in_=W2f[128:256, :])
            b1_sb = cp.tile([128, 256], F16)
            nc.sync.dma_start(out=b1_sb[:], in_=b1rep[:])
            b2_sb = cp.tile([128, 64], F32)
            nc.sync.dma_start(out=b2_sb[:], in_=b2rep[:])
            as2_sb = cp.tile([128, 64], F32)
            nc.sync.dma_start(out=as2_sb[:], in_=as2rep[:])
            ad2_sb = cp.tile([128, 64], F32)
            nc.sync.dma_start(out=ad2_sb[:], in_=ad2rep[:])
            iot_sb = cp.tile([128, 128 * CH], F16)
            nc.sync.dma_start(out=iot_sb[:], in_=iotarep[:])
            pcol_sb = cp.tile([128, 128], F16)
            nc.sync.dma_start(out=pcol_sb[:], in_=pcol[:])
            idn = cp.tile([128, 128], F16)
            make_identity(nc, idn[:])
            g1i_sb = cp.tile([128, T * CH * 8], I16)
            nc.sync.dma_start(out=g1i_sb[:], in_=g1idx[:])
            g2i_sb = cp.tile([128, T * CH * 8], I16)
            nc.sync.dma_start(out=g2i_sb[:], in_=g2idx[:])
            dl1_sb = cp.tile([128, T * CH], F16)
            nc.sync.dma_start(out=dl1_sb[:], in_=dloc1[:])
            dl2_sb = cp.tile([128, T * CH], F16)
            nc.sync.dma_start(out=dl2_sb[:], in_=dloc2[:])
            adrow_sb = cp.tile([128, T], I32)
            nc.sync.dma_start(out=adrow_sb[:], in_=adrow[:])
            t2w_sb = cp.tile([128, T], I32)
            nc.sync.dma_start(out=t2w_sb[:], in_=t2w[:])
            outw_sb = cp.tile([128, T], I32)
            nc.sync.dma_start(out=outw_sb[:], in_=outw[:])
            adst2_sb = cp.tile([128, T], F16)  # written in B, read in D

            # ---------------- Phase A: T1 build (replicated) ----------------
            if "A" in phases:
                with tc.tile_pool(name="pa", bufs=3) as pa, \
                     tc.tile_pool(name="paps", bufs=3, space="PSUM") as paps:
                    XB = 2048  # nodes per xT block (16 tiles)
                    nblk = (NPAD + XB - 1) // XB
                    for blk in range(nblk):
                        n0 = blk * XB
                        bw = min(XB, NPAD - n0)
                        nt = bw // 128
                        xb = pa.tile([128, bw], F16, tag="xb", name="xb")
                        nc.sync.dma_start(out=xb[:], in_=xT[:, n0:n0 + bw])
                        t1b = pa.tile([128, nt, 264], F16, tag="t1b", name="t1b")
                        adb = pa.tile([128, nt, 8], F16, tag="adb", name="adb")
                        for i in range(nt):
                            ps = paps.tile([128, 272], F32, tag="aps", name="aps")
                            nc.tensor.matmul(ps[:],
                                             lhsT=xb[:, i * 128:(i + 1) * 128],
                                             rhs=W1e_sb[:], start=True, stop=True)
                            if i % 2 == 0:
                                nc.scalar.copy(t1b[:, i, :], ps[:, 0:264])
                                nc.vector.tensor_copy(adb[:, i, :], ps[:, 264:272])
                            else:
                                nc.vector.tensor_copy(t1b[:, i, :], ps[:, 0:264])
                                nc.scalar.copy(adb[:, i, :], ps[:, 264:272])
                        nc.sync.dma_start(
                            out=T1[n0:n0 + bw, 0:264].rearrange(
                                "(i p) c -> p i c", p=128),
                            in_=t1b[:])
                        nc.scalar.dma_start(
                            out=adst1[n0:n0 + bw, :].rearrange(
                                "(i p) c -> p i c", p=128),
                            in_=adb[:])

            # ---------------- Phase B: layer-1 aggregation ----------------
            if "B" in phases:
                _agg_layer(nc, tc, T, layer=1,
                           tbl_lo=T1[0:T1_LO, :], tbl_hi=T1[T1_LO:NPAD, :],
                           gidx_sb=g1i_sb, dloc_sb=dl1_sb, dlocT_in=dlocT1,
                           iot_sb=iot_sb, pcol_sb=pcol_sb, idn=idn,
                           adrow_sb=adrow_sb, adst1=adst1,
                           b1_sb=b1_sb, W2a_sb=W2a_sb, W2b_sb=W2b_sb,
                           as2_sb=as2_sb, ad2_sb=ad2_sb, adst2_sb=adst2_sb,
                           t2w_sb=t2w_sb, t2slice=t2slice,
                           b2_sb=None, outw_sb=None, outp=None)

            # ---------------- Phase C: AllGather T2 ----------------
            if "C" in phases:
                if with_collective:
                    nc.gpsimd.collective_compute(
                        "AllGather", A.bypass,
                        replica_groups=[list(range(NCORES))],
                        ins=[t2slice[:]], outs=[t2full[:]],
                    )
                else:
                    nc.sync.dma_start(out=t2full[0:T2_SLICE, :], in_=t2slice[:])

            # ---------------- Phase D: layer-2 aggregation ----------------
            if "D" in phases:
                _agg_layer(nc, tc, T, layer=2,
                           tbl_lo=t2full[0:T2_LO, :],
                           tbl_hi=t2full[T2_LO:T2_FULL, :],
                           gidx_sb=g2i_sb, dloc_sb=dl2_sb, dlocT_in=dlocT2,
                           iot_sb=iot_sb, pcol_sb=pcol_sb, idn=idn,
                           adrow_sb=None, adst1=None,
                           b1_sb=None, W2a_sb=None, W2b_sb=None,
                           as2_sb=None, ad2_sb=None, adst2_sb=adst2_sb,
                           t2w_sb=None, t2slice=None,
                           b2_sb=b2_sb, outw_sb=outw_sb, outp=outp)

    nc.compile()
    return nc


def _agg_layer(nc, tc, T, layer, tbl_lo, tbl_hi, gidx_sb, dloc_sb, dlocT_in,
               iot_sb, pcol_sb, idn, adrow_sb, adst1, b1_sb, W2a_sb, W2b_sb,
               as2_sb, ad2_sb, adst2_sb, t2w_sb, t2slice, b2_sb, outw_sb, outp):
    L1 = layer == 1
    GW = T1_W if L1 else 128  # gathered row width
    WW = 264 if L1 else 65    # w tile width (values + ex columns)
    NH = 8 if L1 else 1       # heads
    ACC_W = 264 if L1 else 65
    name = f"l{layer}"
    with tc.tile_pool(name=f"pb_{name}", bufs=2) as pb, \
         tc.tile_pool(name=f"pf_{name}", bufs=2) as pf, \
         tc.tile_pool(name=f"ps_acc_{name}", bufs=3, space="PSUM") as ps_acc, \
         tc.tile_pool(name=f"ps_ad_{name}", bufs=2, space="PSUM") as ps_ad, \
         tc.tile_pool(name=f"ps_fin_{name}", bufs=1, space="PSUM") as ps_fin:
        for pr in range(T // 2):
            accs = [ps_acc.tile([128, ACC_W], F32, tag="acc", name="acc_a"),
                    ps_acc.tile([128, ACC_W], F32, tag="acc", name="acc_b")]
            if L1:
                adts = []
                for k in (0, 1):
                    t = 2 * pr + k
                    adt = pb.tile([128, 8], F16, tag=f"adt{k}", name="adt")
                    nc.gpsimd.indirect_dma_start(
                        out=adt[:], out_offset=None, in_=adst1,
                        in_offset=bass.IndirectOffsetOnAxis(
                            ap=adrow_sb[:, t:t + 1], axis=0))
                    adts.append(adt)
            for hf in (0, 1):
                g = 2 * pr + hf
                gt = pb.tile([128, CH, GW], F16, tag="gt", name="gt")
                nc.gpsimd.dma_gather(
                    gt[:], tbl_lo if hf == 0 else tbl_hi,
                    gidx_sb[:, g * CH * 8:(g + 1) * CH * 8],
                    CALLW, CALLW, GW, single_packet=False)
                dlT = pb.tile([128, CH, 128], F16, tag="dlT", name="dlT")
                nc.scalar.dma_start(
                    out=dlT[:].rearrange("p j e -> p (j e)"),
                    in_=dlocT_in[:, g * CALLW:(g + 1) * CALLW])
                # sel[e, d, j] (layout [128, 128, CH]): 2x-mode is_equal
                sel = pb.tile([128, 128, CH], F16, tag="sel", name="sel")
                nc.vector.tensor_tensor(
                    out=sel[:],
                    in0=dloc_sb[:, None, g * CH:(g + 1) * CH].to_broadcast(
                        [128, 128, CH]),
                    in1=iot_sb[:].rearrange("p (d j) -> p d j", j=CH),
                    op=A.is_equal)
                # selT[d, j, e]: 2x-mode is_equal
                selT = pb.tile([128, CH, 128], F16, tag="selT", name="selT")
                nc.vector.tensor_tensor(
                    out=selT[:],
                    in0=pcol_sb[:, None, :].to_broadcast([128, CH, 128]),
                    in1=dlT[:],
                    op=A.is_equal)
                # per-edge a_dst via one-hot matmul (segmented broadcast)
                adps = ps_ad.tile([128, CH, NH], F32, tag="adps", name="adps")
                for j in range(CH):
                    t = 2 * pr + (0 if j < CHL else 1)
                    if L1:
                        rhs = adts[0 if j < CHL else 1][:]
                    else:
                        rhs = adst2_sb[:, t:t + 1]
                    nc.tensor.matmul(adps[:, j, :], lhsT=selT[:, j, :], rhs=rhs,
                                     start=True, stop=True)
                adf = pb.tile([128, CH, NH], F16, tag="adf", name="adf")
                nc.scalar.copy(adf[:], adps[:])
                et = pb.tile([128, CH, NH], F16, tag="et", name="et")
                asrc_ap = gt[:, :, 256:264] if L1 else gt[:, :, 64:65]
                nc.vector.tensor_tensor(out=et[:], in0=asrc_ap, in1=adf[:],
                                        op=A.add)
                lk = pb.tile([128, CH, NH], F16, tag="lk", name="lk")
                nc.vector.scalar_tensor_tensor(out=lk[:], in0=et[:], scalar=NEG,
                                               in1=et[:], op0=A.mult, op1=A.max)
                w = pb.tile([128, CH, WW], F16, tag="w", name="w")
                nc.scalar.activation(w[:, :, WW - NH:WW], lk[:], AF.Exp)
                if L1:
                    # heads 4-7: ACT-expanded ex, then flat 2x multiply
                    exx = pb.tile([128, CH, 128], F16, tag="exx", name="exx")
                    nc.scalar.activation(
                        exx[:].rearrange("p j (h c) -> p j h c", h=4),
                        lk[:, :, 4:8][:, :, :, None].to_broadcast(
                            [128, CH, 4, 32]),
                        AF.Exp)
                    nc.vector.tensor_tensor(
                        out=w[:, :, 128:256], in0=gt[:, :, 128:256],
                        in1=exx[:], op=A.mult)
                    # heads 0-3: broadcast multiply (1x)
                    nc.vector.tensor_tensor(
                        out=w[:, :, 0:128].rearrange("p j (h c) -> p j h c", h=4),
                        in0=gt[:, :, 0:128].rearrange("p j (h c) -> p j h c", h=4),
                        in1=w[:, :, 256:260][:, :, :, None].to_broadcast(
                            [128, CH, 4, 32]),
                        op=A.mult)
                else:
                    exx = pb.tile([128, CH, 64], F16, tag="exx", name="exx")
                    nc.scalar.activation(
                        exx[:],
                        lk[:][:, :, :].to_broadcast([128, CH, 64]),
                        AF.Exp)
                    nc.vector.tensor_tensor(
                        out=w[:, :, 0:64], in0=gt[:, :, 0:64],
                        in1=exx[:], op=A.mult)
                for j in range(CH):
                    acc = accs[0 if j < CHL else 1]
                    st = (hf == 0) and (j % CHL == 0)
                    sp = (hf == 1) and (j % CHL == CHL - 1)
                    nc.tensor.matmul(acc[:], lhsT=sel[:, :, j], rhs=w[:, j, :],
                                     start=st, stop=sp)
            for k in (0, 1):
                t = 2 * pr + k
                if L1:
                    _fin_l1(nc, t, accs[k], pf, ps_fin, idn, b1_sb, W2a_sb,
                            W2b_sb, as2_sb, ad2_sb, adst2_sb, t2w_sb, t2slice)
                else:
                    _fin_l2(nc, t, accs[k], pf, b2_sb, outw_sb, outp)


def _fin_l1(nc, t, acc, pf, ps_fin, idn, b1_sb, W2a_sb, W2b_sb, as2_sb,
            ad2_sb, adst2_sb, t2w_sb, t2slice):
    deps = pf.tile([128, 8], F32, tag="deps", name="deps")
    nc.vector.tensor_scalar_add(deps[:], acc[:, 256:264], EPS)
    rec = pf.tile([128, 8], F32, tag="rec", name="rec")
    nc.vector.reciprocal(rec[:], deps[:])
    h1b = pf.tile([128, 256], F16, tag="h1b", name="h1b")
    nc.vector.tensor_tensor(
        out=h1b[:].rearrange("p (h c) -> p h c", h=8),
        in0=acc[:, 0:256].rearrange("p (h c) -> p h c", h=8),
        in1=rec[:, :, None].to_broadcast([128, 8, 32]),
        op=A.mult)
    nc.vector.tensor_tensor(out=h1b[:], in0=h1b[:], in1=b1_sb[:], op=A.add)
    # elu(x) = relu(x) + exp(-relu(-x)) - 1   (ACT-heavy)
    r1 = pf.tile([128, 256], F16, tag="r1", name="r1")
    nc.scalar.activation(r1[:], h1b[:], AF.Relu, scale=-1.0)
    em = pf.tile([128, 256], F16, tag="em", name="em")
    nc.scalar.activation(em[:], r1[:], AF.Exp, scale=-1.0)
    rl = pf.tile([128, 256], F16, tag="rl", name="rl")
    nc.scalar.activation(rl[:], h1b[:], AF.Relu)
    ho = pf.tile([128, 256], F16, tag="ho", name="ho")
    nc.vector.scalar_tensor_tensor(out=ho[:], in0=rl[:], scalar=-1.0,
                                   in1=em[:], op0=A.add, op1=A.add)
    # h2 = ho @ W2 via two transposed matmuls
    h2ps = ps_fin.tile([128, 64], F32, tag="h2ps", name="h2ps")
    for half in (0, 1):
        tp = ps_fin.tile([128, 128], F16, tag="tp", name="tp")
        nc.tensor.transpose(out=tp[:], in_=ho[:, half * 128:(half + 1) * 128],
                            identity=idn[:])
        hoT = pf.tile([128, 128], F16, tag="hoT", name="hoT")
        nc.scalar.copy(hoT[:], tp[:])
        nc.tensor.matmul(h2ps[:], lhsT=hoT[:],
                         rhs=(W2a_sb if half == 0 else W2b_sb)[:],
                         start=half == 0, stop=half == 1)
    t2r = pf.tile([128, 128], F16, tag="t2r", name="t2r")
    nc.scalar.copy(t2r[:, 0:64], h2ps[:])
    tmp = pf.tile([128, 64], F32, tag="tmp", name="tmp")
    nc.vector.tensor_tensor(out=tmp[:], in0=h2ps[:], in1=as2_sb[:], op=A.mult)
    a2s = pf.tile([128, 1], F32, tag="a2s", name="a2s")
    nc.vector.reduce_sum(a2s[:], tmp[:], axis=mybir.AxisListType.X)
    nc.vector.tensor_copy(t2r[:, 64:65], a2s[:])
    nc.vector.tensor_tensor(out=tmp[:], in0=h2ps[:], in1=ad2_sb[:], op=A.mult)
    a2d = pf.tile([128, 1], F32, tag="a2d", name="a2d")
    nc.vector.reduce_sum(a2d[:], tmp[:], axis=mybir.AxisListType.X)
    nc.vector.tensor_copy(adst2_sb[:, t:t + 1], a2d[:])
    nc.gpsimd.indirect_dma_start(
        out=t2slice, out_offset=bass.IndirectOffsetOnAxis(
            ap=t2w_sb[:, t:t + 1], axis=0),
        in_=t2r[:], in_offset=None)


def _fin_l2(nc, t, acc, pf, b2_sb, outw_sb, outp):
    dep = pf.tile([128, 1], F32, tag="dep2", name="dep2")
    nc.vector.tensor_scalar_add(dep[:], acc[:, 64:65], EPS)
    rec = pf.tile([128, 1], F32, tag="rec2", name="rec2")
    nc.vector.reciprocal(rec[:], dep[:])
    ot = pf.tile([128, 64], F32, tag="ot", name="ot")
    nc.vector.tensor_scalar_mul(ot[:], acc[:, 0:64], rec[:, 0:1])
    nc.vector.tensor_tensor(out=ot[:], in0=ot[:], in1=b2_sb[:], op=A.add)
    nc.gpsimd.indirect_dma_start(
        out=outp, out_offset=bass.IndirectOffsetOnAxis(
            ap=outw_sb[:, t:t + 1], axis=0),
        in_=ot[:], in_offset=None)


# ---------------------------------------------------------------------------
# entry point
# ---------------------------------------------------------------------------

def make_in_maps(T, common, per_core):
    in_maps = []
    for c in range(NCORES):
        m = {
            "xT": common["xT"], "W1ext": common["W1ext"], "W2f": common["W2f"],
            "b1rep": common["b1rep"], "b2rep": common["b2rep"],
            "as2rep": common["as2rep"], "ad2rep": common["ad2rep"],
            "iotarep": common["iotarep"], "pcol": common["pcol"],
        }
        m.update(per_core[c])
        in_maps.append(m)
    return in_maps


def kernel(**inputs):
    T, common, per_core = host_prep(inputs)
    nc = build_nc(T)
    in_maps = make_in_maps(T, common, per_core)
    res = run_bass_kernel_spmd(nc, in_maps, core_ids=list(range(NCORES)))
    out = np.concatenate([res.results[c]["out"][:NPC] for c in range(NCORES)],
                         axis=0)
    return out.astype(np.float32)


# revision 22
# speedup vs baseline: 2.5386x; 2.0053x over previous
"""Two-layer GAT on 8 Trainium2 NeuronCores (Bass/Tile).

Strategy (edge-cut, dst-sharded):
  - Core c owns destination nodes [c*6250, (c+1)*6250).
  - Phase A (replicated): every core computes the full layer-1 feature table
    T1[n] = [h1(256) | a_src1(8)] = x @ [W1 | W1@As | W1@Ad] (fp16), plus
    a_dst1[n], written to local HBM.  No collectives.
  - Phase B: per-core edge aggregation for its own dst nodes.  Edges sorted
    into (<=128-dst-node) tiles; per 128-edge chunk, gather T1[src] rows via
    dma_gather, compute per-edge ex = exp(leaky(a_src+a_dst)), and
    scatter-add via a one-hot (selection-matrix) matmul into PSUM.
    Softmax normalization happens per dst tile at finalize, followed by
    bias+elu and the layer-2 input transform h2 = h @ W2, producing the
    layer-2 table slice T2[own] = [h2(64) | a_src2(1)].
  - Phase C: AllGather of T2 slices across the 8 cores.
  - Phase D: same aggregation machinery for layer 2 (single head), writing
    the final output rows.

dma_gather uses int16 indices, so the gather tables are addressed in two
halves (lo/hi row slices); each 128-edge chunk's edges come from a single
half (host-side grouping).  Selection matrices are built on the DVE in
layouts where every operand streams at unit stride (2x perf mode).
"""

import numpy as np

import concourse.bass as bass
import concourse.bacc as bacc
import concourse.tile as tile
import concourse.mybir as mybir
from concourse.bass_utils import run_bass_kernel_spmd
from concourse.masks import make_identity

F32 = mybir.dt.float32
F16 = mybir.dt.float16
I8 = mybir.dt.int8
I16 = mybir.dt.int16
I32 = mybir.dt.int32
A = mybir.AluOpType
AF = mybir.ActivationFunctionType

# -------- problem constants (hardcoded per the task contract) --------
N, E, IN, HID, OUT, H = 50000, 800000, 128, 32, 64, 8
C1 = H * HID  # 256
NCORES = 8
NPC = N // NCORES  # 6250 dst nodes per core
NTILE_A = 391  # ceil(50000/128)
NPAD = NTILE_A * 128  # 50048
T1_LO = 25088  # row split of T1 (int16 index range)
T1_W = 384  # T1 row width (768B, multiple of 256B): h(256)+asrc(8)+pad
# T2/output are slot-major: row = tile*128 + slot (T runtime-dependent)
CHL = 9  # chunks per (tile, table-half)
CH = 2 * CHL  # chunks per gather call (two tiles' worth of one half)
CALLW = CH * 128  # gather rows per call
EPS = 1e-16
NEG = 0.2


# ---------------------------------------------------------------------------
# host-side preprocessing
# ---------------------------------------------------------------------------

def _prep_weights(W1, as1, ad1, b1, W2, as2, ad2, b2):
    As = np.zeros((C1, H), np.float32)
    Ad = np.zeros((C1, H), np.float32)
    for h in range(H):
        As[h * HID:(h + 1) * HID, h] = as1[h]
        Ad[h * HID:(h + 1) * HID, h] = ad1[h]
    W1ext = np.concatenate([W1, W1 @ As, W1 @ Ad], axis=1)  # [128, 272]
    iotarep = np.tile(np.arange(128, dtype=np.float16)[:, None],
                      (1, CH)).reshape(1, 128 * CH)
    return {
        "W1ext": W1ext.astype(np.float16),
        "W2f": W2.astype(np.float16),  # [256, 64]
        "b1rep": np.tile(b1[None, :], (128, 1)).astype(np.float16),
        "b2rep": np.tile(b2[None, :], (128, 1)).astype(np.float32),
        "as2rep": np.tile(as2[0][None, :], (128, 1)).astype(np.float32),
        "ad2rep": np.tile(ad2[0][None, :], (128, 1)).astype(np.float32),
        # iotarep[p, d*CH + j] = d
        "iotarep": np.tile(iotarep, (128, 1)),
        # pcol[p, e] = p
        "pcol": np.tile(np.arange(128, dtype=np.float16)[:, None], (1, 128)),
    }


def _greedy_tiles(deg_lo1, deg_hi1, deg_lo2, deg_hi2):
    """Pack the core's 6250 nodes into tiles of width<=128 with each of the
    four per-half edge counts <= CHL*128."""
    cap = CHL * 128
    tiles = []  # (n0, n1) local node ranges
    i, n = 0, len(deg_lo1)
    while i < n:
        l1 = h1 = l2 = h2 = 0
        j = i
        while j < n and j - i < 128:
            nl1, nh1 = l1 + deg_lo1[j], h1 + deg_hi1[j]
            nl2, nh2 = l2 + deg_lo2[j], h2 + deg_hi2[j]
            if nl1 > cap or nh1 > cap or nl2 > cap or nh2 > cap:
                break
            l1, h1, l2, h2 = nl1, nh1, nl2, nh2
            j += 1
        assert j > i, "single node exceeds chunk caps"
        tiles.append((i, j))
        i = j
    return tiles


def _pack_calls(rows_per_tile_half, dloc_per_tile_half, T, tdt=np.float16):
    """Build gather-call arrays.

    Call g = (pair pr, half h) covers tiles 2pr (chunks 0..CHL-1) and 2pr+1
    (chunks CHL..CH-1).  Edge position i in a call lands on SBUF partition
    i%128, chunk i//128 (dma_gather layout), with the int16 index read from
    idx16[i%16, g*CH*8 + i//16].

    Returns idx16 [128, T*CH*8] i16, dloc [128, T*CH] f16,
    dlocT [128, T*CH*128] f16 (partition-replicated transpose).
    """
    ncalls = T
    idx16 = np.zeros((128, ncalls * CH * 8), np.int16)
    dloc = np.full((128, ncalls * CH), -1, np.float16)
    dlocT_flat = np.full((ncalls * CH * 128,), -1, tdt)
    for pr in range(T // 2):
        for h in (0, 1):
            g = 2 * pr + h
            rows = np.zeros((CALLW,), np.int64)
            dl = np.full((CALLW,), -1, np.int64)
            for k, t in enumerate((2 * pr, 2 * pr + 1)):
                r = rows_per_tile_half.get((t, h))
                if r is None:
                    continue
                d = dloc_per_tile_half[(t, h)]
                off = k * CHL * 128
                rows[off:off + len(r)] = r
                dl[off:off + len(r)] = d
            blk = rows.reshape(CH * 8, 16).T.astype(np.int16)  # [16, CH*8]
            idx16[:, g * CH * 8:(g + 1) * CH * 8] = np.tile(blk, (8, 1))
            dloc[:, g * CH:(g + 1) * CH] = dl.reshape(CH, 128).T.astype(np.float16)
            dlocT_flat[g * CALLW:(g + 1) * CALLW] = dl.astype(np.float16)
    dlocT = np.tile(dlocT_flat[None, :], (128, 1))
    return idx16, dloc, dlocT


def _prep_core(c, src, dst):
    base = c * NPC
    own = (dst >= base) & (dst < base + NPC)
    s = src[own].astype(np.int64)
    d = (dst[own] - base).astype(np.int64)
    order = np.argsort(d, kind="stable")
    s, d = s[order], d[order]
    ptr = np.zeros(NPC + 1, np.int64)
    np.cumsum(np.bincount(d, minlength=NPC), out=ptr[1:])

    lo1m = s < T1_LO
    lo2m = s < 25000
    deg_lo1 = np.bincount(d, weights=lo1m, minlength=NPC).astype(np.int64)
    deg_hi1 = np.bincount(d, weights=~lo1m, minlength=NPC).astype(np.int64)
    deg_lo2 = np.bincount(d, weights=lo2m, minlength=NPC).astype(np.int64)
    deg_hi2 = np.bincount(d, weights=~lo2m, minlength=NPC).astype(np.int64)
    tiles = _greedy_tiles(deg_lo1, deg_hi1, deg_lo2, deg_hi2)

    rows1, dloc1 = {}, {}
    for t, (n0, n1) in enumerate(tiles):
        e0, e1 = ptr[n0], ptr[n1]
        es, ed = s[e0:e1], d[e0:e1]
        dl = ed - n0
        m1 = es < T1_LO
        rows1[(t, 0)] = es[m1]
        dloc1[(t, 0)] = dl[m1]
        rows1[(t, 1)] = es[~m1] - T1_LO
        dloc1[(t, 1)] = dl[~m1]

    return {
        "Treal": len(tiles), "tiles": tiles, "s": s, "d": d, "ptr": ptr,
        "rows1": rows1, "dloc1": dloc1,
    }


def _finish_core(pc, c, T, slotrow):
    tiles = list(pc["tiles"]) + [(0, 0)] * (T - pc["Treal"])
    idx1, dloc1, dlocT1 = _pack_calls(pc["rows1"], pc["dloc1"], T)

    # layer-2 gather rows in slot-major space (lo/hi split at 4 cores)
    t2lo = 4 * T * 128
    s, d, ptr = pc["s"], pc["d"], pc["ptr"]
    srow = slotrow[s]
    rows2, dloc2 = {}, {}
    for t, (n0, n1) in enumerate(pc["tiles"]):
        e0, e1 = ptr[n0], ptr[n1]
        dl = d[e0:e1] - n0
        m2 = srow[e0:e1] < t2lo
        rows2[(t, 0)] = srow[e0:e1][m2]
        dloc2[(t, 0)] = dl[m2]
        rows2[(t, 1)] = srow[e0:e1][~m2] - t2lo
        dloc2[(t, 1)] = dl[~m2]
    idx2, dloc2a, dlocT2 = _pack_calls(rows2, dloc2, T, tdt=np.int8)

    base = c * NPC
    adrow = np.zeros((128, T), np.int32)
    p = np.arange(128)
    for t, (n0, n1) in enumerate(tiles):
        w = n1 - n0
        adrow[:, t] = base + n0 + np.minimum(p, max(w - 1, 0))
    return {
        "g1idx": idx1, "dloc1": dloc1, "dlocT1": dlocT1,
        "g2idx": idx2, "dloc2": dloc2a, "dlocT2": dlocT2,
        "adrow": adrow,
    }


def host_prep(inputs):
    ei = np.asarray(inputs["edge_index"]).astype(np.int64)
    wd = _prep_weights(
        np.asarray(inputs["W1"], np.float32),
        np.asarray(inputs["att_src1"], np.float32),
        np.asarray(inputs["att_dst1"], np.float32),
        np.asarray(inputs["b1"], np.float32),
        np.asarray(inputs["W2"], np.float32),
        np.asarray(inputs["att_src2"], np.float32),
        np.asarray(inputs["att_dst2"], np.float32),
        np.asarray(inputs["b2"], np.float32),
    )
    loops = np.arange(N, dtype=np.int64)
    src = np.concatenate([ei[0], loops])
    dst = np.concatenate([ei[1], loops])

    xT = np.zeros((IN, NPAD), np.float16)
    xT[:, :N] = np.asarray(inputs["x"], np.float32).T.astype(np.float16)

    cores = [_prep_core(c, src, dst) for c in range(NCORES)]
    T = max(pc["Treal"] for pc in cores)
    if T % 2:
        T += 1
    # slotrow[n]: row of node n in the slot-major T2_full / per-core output
    slotrow = np.zeros(N, np.int64)
    for c, pc in enumerate(cores):
        base_row = c * T * 128
        for t, (n0, n1) in enumerate(pc["tiles"]):
            nodes = c * NPC + np.arange(n0, n1)
            slotrow[nodes] = base_row + t * 128 + np.arange(n1 - n0)
    per_core = [_finish_core(pc, c, T, slotrow) for c, pc in enumerate(cores)]

    common = dict(wd)
    common["xT"] = xT
    common["slotrow"] = slotrow  # host-side output reorder
    return T, common, per_core


# ---------------------------------------------------------------------------
# device program
# ---------------------------------------------------------------------------

def build_nc(T, num_devices=NCORES, with_collective=True, phases="ABCD",
             dbg=False):
    nc = bacc.Bacc("TRN2", target_bir_lowering=False, debug=False,
                   num_devices=num_devices)
    dt = nc.dram_tensor
    xT = dt("xT", [IN, NPAD], F16, kind="ExternalInput").ap()
    W1ext = dt("W1ext", [128, 272], F16, kind="ExternalInput").ap()
    W2f = dt("W2f", [256, 64], F16, kind="ExternalInput").ap()
    b1rep = dt("b1rep", [128, 256], F16, kind="ExternalInput").ap()
    b2rep = dt("b2rep", [128, 64], F32, kind="ExternalInput").ap()
    as2rep = dt("as2rep", [128, 64], F32, kind="ExternalInput").ap()
    ad2rep = dt("ad2rep", [128, 64], F32, kind="ExternalInput").ap()
    iotarep = dt("iotarep", [128, 128 * CH], F16, kind="ExternalInput").ap()
    pcol = dt("pcol", [128, 128], F16, kind="ExternalInput").ap()
    g1idx = dt("g1idx", [128, T * CH * 8], I16, kind="ExternalInput").ap()
    g2idx = dt("g2idx", [128, T * CH * 8], I16, kind="ExternalInput").ap()
    dloc1 = dt("dloc1", [128, T * CH], F16, kind="ExternalInput").ap()
    dloc2 = dt("dloc2", [128, T * CH], F16, kind="ExternalInput").ap()
    dlocT1 = dt("dlocT1", [128, T * CH * 128], F16, kind="ExternalInput").ap()
    dlocT2 = dt("dlocT2", [128, T * CH * 128], I8, kind="ExternalInput").ap()
    adrow = dt("adrow", [128, T], I32, kind="ExternalInput").ap()
    T1 = dt("T1", [NPAD, T1_W], F16, kind="Internal").ap()
    t2rows = T * 128
    t2slice = dt("t2slice", [t2rows, 128], F16, kind="Internal").ap()
    t2full = dt("t2full", [NCORES * t2rows, 128], F16, kind="Internal",
                addr_space="Shared" if with_collective else "Local").ap()
    outp = dt("out", [t2rows, 64], F32, kind="ExternalOutput").ap()
    if dbg:
        t2dbg = dt("t2dbg", [NCORES * t2rows, 128], F16,
                   kind="ExternalOutput").ap()

    with tile.TileContext(nc) as tc:
        with tc.tile_pool(name="consts", bufs=1) as cp:
            W1e_sb = cp.tile([128, 272], F16)
            nc.sync.dma_start(out=W1e_sb[:], in_=W1ext[:])
            W2a_sb = cp.tile([128, 64], F16)
            nc.sync.dma_start(out=W2a_sb[:], in_=W2f[0:128, :])
            W2b_sb = cp.tile([128, 64], F16)
            nc.sync.dma_start(out=W2b_sb[:], in_=W2f[128:256, :])
            b1_sb = cp.tile([128, 256], F16)
            nc.sync.dma_start(out=b1_sb[:], in_=b1rep[:])
            b2_sb = cp.tile([128, 64], F32)
            nc.sync.dma_start(out=b2_sb[:], in_=b2rep[:])
            as2_sb = cp.tile([128, 64], F32)
            nc.sync.dma_start(out=as2_sb[:], in_=as2rep[:])
            ad2_sb = cp.tile([128, 64], F32)
            nc.sync.dma_start(out=ad2_sb[:], in_=ad2rep[:])
            iot_sb = cp.tile([128, 128 * CH], F16)
            nc.sync.dma_start(out=iot_sb[:], in_=iotarep[:])
            pcol_sb = cp.tile([128, 128], F16)
            nc.sync.dma_start(out=pcol_sb[:], in_=pcol[:])
            idn = cp.tile([128, 128], F16)
            make_identity(nc, idn[:])
            g1i_sb = cp.tile([128, T * CH * 8], I16)
            nc.sync.dma_start(out=g1i_sb[:], in_=g1idx[:])
            g2i_sb = cp.tile([128, T * CH * 8], I16)
            nc.sync.dma_start(out=g2i_sb[:], in_=g2idx[:])
            dl1_sb = cp.tile([128, T * CH], F16)
            nc.sync.dma_start(out=dl1_sb[:], in_=dloc1[:])
            dl2_sb = cp.tile([128, T * CH], F16)
            nc.sync.dma_start(out=dl2_sb[:], in_=dloc2[:])
            adrow_sb = cp.tile([128, T], I32)
            nc.sync.dma_start(out=adrow_sb[:], in_=adrow[:])
            adst2_sb = cp.tile([128, T], F16)  # written in B, read in D

            # ---------------- Phase A: T1 build (replicated) ----------------
            if "A" in phases:
                with tc.tile_pool(name="pa", bufs=3) as pa, \
                     tc.tile_pool(name="paps", bufs=3, space="PSUM") as paps:
                    XB = 2048  # nodes per xT block (16 tiles)
                    nblk = (NPAD + XB - 1) // XB
                    for blk in range(nblk):
                        n0 = blk * XB
                        bw = min(XB, NPAD - n0)
                        nt = bw // 128
                        xb = pa.tile([128, bw], F16, tag="xb", name="xb")
                        nc.sync.dma_start(out=xb[:], in_=xT[:, n0:n0 + bw])
                        t1b = pa.tile([128, nt, 272], F16, tag="t1b", name="t1b")
                        for i in range(nt):
                            ps = paps.tile([128, 272], F32, tag="aps", name="aps")
                            nc.tensor.matmul(ps[:],
                                             lhsT=xb[:, i * 128:(i + 1) * 128],
                                             rhs=W1e_sb[:], start=True, stop=True)
                            if i % 2 == 0:
                                nc.scalar.copy(t1b[:, i, :], ps[:])
                            else:
                                nc.vector.tensor_copy(t1b[:, i, :], ps[:])
                        nc.sync.dma_start(
                            out=T1[n0:n0 + bw, 0:272].rearrange(
                                "(i p) c -> p i c", p=128),
                            in_=t1b[:])

            # ---------------- Phase B: layer-1 aggregation ----------------
            if "B" in phases:
                _agg_layer(nc, tc, T, layer=1,
                           tbl_lo=T1[0:T1_LO, :], tbl_hi=T1[T1_LO:NPAD, :],
                           gidx_sb=g1i_sb, dloc_sb=dl1_sb, dlocT_in=dlocT1,
                           iot_sb=iot_sb, pcol_sb=pcol_sb, idn=idn,
                           adrow_sb=adrow_sb, adst1=T1[:],
                           b1_sb=b1_sb, W2a_sb=W2a_sb, W2b_sb=W2b_sb,
                           as2_sb=as2_sb, ad2_sb=ad2_sb, adst2_sb=adst2_sb,
                           t2slice=t2slice, b2_sb=None, outp=None)

            # ---------------- Phase C: AllGather T2 ----------------
            if "C" in phases:
                if with_collective:
                    nc.gpsimd.collective_compute(
                        "AllGather", A.bypass,
                        replica_groups=[list(range(NCORES))],
                        ins=[t2slice[:]], outs=[t2full[:]],
                    )
                else:
                    nc.sync.dma_start(out=t2full[0:t2rows, :], in_=t2slice[:])

            if dbg:
                nc.sync.dma_start(out=t2dbg[:], in_=t2full[:])

            # ---------------- Phase D: layer-2 aggregation ----------------
            if "D" in phases:
                _agg_layer(nc, tc, T, layer=2,
                           tbl_lo=t2full[0:4 * t2rows, :],
                           tbl_hi=t2full[4 * t2rows:8 * t2rows, :],
                           gidx_sb=g2i_sb, dloc_sb=dl2_sb, dlocT_in=dlocT2,
                           iot_sb=iot_sb, pcol_sb=pcol_sb, idn=idn,
                           adrow_sb=None, adst1=None,
                           b1_sb=None, W2a_sb=None, W2b_sb=None,
                           as2_sb=None, ad2_sb=None, adst2_sb=adst2_sb,
                           t2slice=None, b2_sb=b2_sb, outp=outp)

    nc.compile()
    return nc


def _agg_layer(nc, tc, T, layer, tbl_lo, tbl_hi, gidx_sb, dloc_sb, dlocT_in,
               iot_sb, pcol_sb, idn, adrow_sb, adst1, b1_sb, W2a_sb, W2b_sb,
               as2_sb, ad2_sb, adst2_sb, t2slice, b2_sb, outp):
    L1 = layer == 1
    GW = T1_W if L1 else 128  # gathered row width
    WW = 264 if L1 else 65    # w tile width (values + ex columns)
    NH = 8 if L1 else 1       # heads
    ACC_W = 264 if L1 else 65
    name = f"l{layer}"
    with tc.tile_pool(name=f"pb_{name}", bufs=2 if L1 else 3) as pb, \
         tc.tile_pool(name=f"pf_{name}", bufs=2) as pf, \
         tc.tile_pool(name=f"ps_acc_{name}", bufs=4, space="PSUM") as ps_acc, \
         tc.tile_pool(name=f"ps_ad_{name}", bufs=2, space="PSUM") as ps_ad, \
         tc.tile_pool(name=f"ps_fin_{name}", bufs=1, space="PSUM") as ps_fin:
        for pr in range(T // 2):
            accs = [ps_acc.tile([128, ACC_W], F32, tag="acc", name="acc_a"),
                    ps_acc.tile([128, ACC_W], F32, tag="acc", name="acc_b")]
            if L1:
                adts = []
                for k in (0, 1):
                    t = 2 * pr + k
                    adt = pb.tile([128, 8], F16, tag=f"adt{k}", name="adt")
                    nc.gpsimd.indirect_dma_start(
                        out=adt[:], out_offset=None, in_=adst1,
                        in_offset=bass.IndirectOffsetOnAxis(
                            ap=adrow_sb[:, t:t + 1], axis=0),
                        element_offset=264)
                    adts.append(adt)
            for hf in (0, 1):
                g = 2 * pr + hf
                gt = pb.tile([128, CH, GW], F16, tag="gt", name="gt", bufs=3)
                nc.gpsimd.dma_gather(
                    gt[:], tbl_lo if hf == 0 else tbl_hi,
                    gidx_sb[:, g * CH * 8:(g + 1) * CH * 8],
                    CALLW, CALLW, GW, single_packet=False)
                dlT = pb.tile([128, CH, 128], F16 if L1 else I8, tag="dlT", name="dlT")
                nc.scalar.dma_start(
                    out=dlT[:].rearrange("p j e -> p (j e)"),
                    in_=dlocT_in[:, g * CALLW:(g + 1) * CALLW])
                # sel[e, d, j] (layout [128, 128, CH]): 2x-mode is_equal
                sel = pb.tile([128, 128, CH], F16, tag="sel", name="sel")
                nc.vector.tensor_tensor(
                    out=sel[:],
                    in0=dloc_sb[:, None, g * CH:(g + 1) * CH].to_broadcast(
                        [128, 128, CH]),
                    in1=iot_sb[:].rearrange("p (d j) -> p d j", j=CH),
                    op=A.is_equal)
                # selT[d, j, e]: 2x-mode is_equal
                selT = pb.tile([128, CH, 128], F16, tag="selT", name="selT")
                nc.vector.tensor_tensor(
                    out=selT[:],
                    in0=pcol_sb[:, None, :].to_broadcast([128, CH, 128]),
                    in1=dlT[:],
                    op=A.is_equal)
                # per-edge a_dst via one-hot matmul (segmented broadcast)
                adps = ps_ad.tile([128, CH, NH], F32, tag="adps", name="adps")
                for j in range(CH):
                    t = 2 * pr + (0 if j < CHL else 1)
                    if L1:
                        rhs = adts[0 if j < CHL else 1][:]
                    else:
                        rhs = adst2_sb[:, t:t + 1]
                    nc.tensor.matmul(adps[:, j, :], lhsT=selT[:, j, :], rhs=rhs,
                                     start=True, stop=True)
                adf = pb.tile([128, CH, NH], F16, tag="adf", name="adf")
                nc.scalar.copy(adf[:], adps[:])
                et = pb.tile([128, CH, NH], F16, tag="et", name="et")
                asrc_ap = gt[:, :, 256:264] if L1 else gt[:, :, 64:65]
                nc.vector.tensor_tensor(out=et[:], in0=asrc_ap, in1=adf[:],
                                        op=A.add)
                lk = pb.tile([128, CH, NH], F16, tag="lk", name="lk")
                nc.vector.scalar_tensor_tensor(out=lk[:], in0=et[:], scalar=NEG,
                                               in1=et[:], op0=A.mult, op1=A.max)
                w = pb.tile([128, CH, WW], F16, tag="w", name="w")
                VC = 256 if L1 else 64  # value columns
                exx = pb.tile([128, CH, VC], F16, tag="exx", name="exx")
                if L1:
                    nc.scalar.activation(
                        exx[:].rearrange("p j (h c) -> p j h c", h=8),
                        lk[:, :, :, None].to_broadcast([128, CH, 8, 32]),
                        AF.Exp)
                    # den columns: every 32nd element of exx
                    nc.vector.tensor_copy(
                        w[:, :, 256:264],
                        exx[:].rearrange("p j (h c) -> p j h c", h=8)[:, :, :, 0])
                else:
                    nc.scalar.activation(
                        exx[:], lk[:][:, :, :].to_broadcast([128, CH, 64]),
                        AF.Exp)
                    nc.vector.tensor_copy(w[:, :, 64:65], exx[:, :, 0:1])
                nc.vector.tensor_tensor(
                    out=w[:, :, 0:VC], in0=gt[:, :, 0:VC], in1=exx[:],
                    op=A.mult)
                for j in range(CH):
                    acc = accs[0 if j < CHL else 1]
                    st = (hf == 0) and (j % CHL == 0)
                    sp = (hf == 1) and (j % CHL == CHL - 1)
                    nc.tensor.matmul(acc[:], lhsT=sel[:, :, j], rhs=w[:, j, :],
                                     start=st, stop=sp)
            for k in (0, 1):
                t = 2 * pr + k
                if L1:
                    _fin_l1(nc, t, accs[k], pf, ps_fin, idn, b1_sb, W2a_sb,
                            W2b_sb, as2_sb, ad2_sb, adst2_sb, t2slice)
                else:
                    _fin_l2(nc, t, accs[k], pf, b2_sb, outp)


def _fin_l1(nc, t, acc, pf, ps_fin, idn, b1_sb, W2a_sb, W2b_sb, as2_sb,
            ad2_sb, adst2_sb, t2slice):
    deps = pf.tile([128, 8], F32, tag="deps", name="deps")
    nc.vector.tensor_scalar_add(deps[:], acc[:, 256:264], EPS)
    rec = pf.tile([128, 8], F32, tag="rec", name="rec")
    nc.vector.reciprocal(rec[:], deps[:])
    h1b = pf.tile([128, 256], F16, tag="h1b", name="h1b")
    nc.vector.tensor_tensor(
        out=h1b[:].rearrange("p (h c) -> p h c", h=8),
        in0=acc[:, 0:256].rearrange("p (h c) -> p h c", h=8),
        in1=rec[:, :, None].to_broadcast([128, 8, 32]),
        op=A.mult)
    nc.vector.tensor_tensor(out=h1b[:], in0=h1b[:], in1=b1_sb[:], op=A.add)
    # elu(x) = relu(x) + exp(-relu(-x)) - 1   (ACT-heavy)
    r1 = pf.tile([128, 256], F16, tag="r1", name="r1")
    nc.scalar.activation(r1[:], h1b[:], AF.Relu, scale=-1.0)
    em = pf.tile([128, 256], F16, tag="em", name="em")
    nc.scalar.activation(em[:], r1[:], AF.Exp, scale=-1.0)
    rl = pf.tile([128, 256], F16, tag="rl", name="rl")
    nc.scalar.activation(rl[:], h1b[:], AF.Relu)
    ho = pf.tile([128, 256], F16, tag="ho", name="ho")
    nc.vector.scalar_tensor_tensor(out=ho[:], in0=rl[:], scalar=-1.0,
                                   in1=em[:], op0=A.add, op1=A.add)
    # h2 = ho @ W2 via two transposed matmuls
    h2ps = ps_fin.tile([128, 64], F32, tag="h2ps", name="h2ps")
    for half in (0, 1):
        tp = ps_fin.tile([128, 128], F16, tag="tp", name="tp")
        nc.tensor.transpose(out=tp[:], in_=ho[:, half * 128:(half + 1) * 128],
                            identity=idn[:])
        hoT = pf.tile([128, 128], F16, tag="hoT", name="hoT")
        nc.scalar.copy(hoT[:], tp[:])
        nc.tensor.matmul(h2ps[:], lhsT=hoT[:],
                         rhs=(W2a_sb if half == 0 else W2b_sb)[:],
                         start=half == 0, stop=half == 1)
    t2r = pf.tile([128, 128], F16, tag="t2r", name="t2r")
    nc.scalar.copy(t2r[:, 0:64], h2ps[:])
    tmp = pf.tile([128, 64], F32, tag="tmp", name="tmp")
    nc.vector.tensor_tensor(out=tmp[:], in0=h2ps[:], in1=as2_sb[:], op=A.mult)
    a2s = pf.tile([128, 1], F32, tag="a2s", name="a2s")
    nc.vector.reduce_sum(a2s[:], tmp[:], axis=mybir.AxisListType.X)
    nc.vector.tensor_copy(t2r[:, 64:65], a2s[:])
    nc.vector.tensor_tensor(out=tmp[:], in0=h2ps[:], in1=ad2_sb[:], op=A.mult)
    a2d = pf.tile([128, 1], F32, tag="a2d", name="a2d")
    nc.vector.reduce_sum(a2d[:], tmp[:], axis=mybir.AxisListType.X)
    nc.vector.tensor_copy(adst2_sb[:, t:t + 1], a2d[:])
    nc.sync.dma_start(out=t2slice[t * 128:(t + 1) * 128, :], in_=t2r[:])


def _fin_l2(nc, t, acc, pf, b2_sb, outp):
    dep = pf.tile([128, 1], F32, tag="dep2", name="dep2")
    nc.vector.tensor_scalar_add(dep[:], acc[:, 64:65], EPS)
    rec = pf.tile([128, 1], F32, tag="rec2", name="rec2")
    nc.vector.reciprocal(rec[:], dep[:])
    ot = pf.tile([128, 64], F32, tag="ot", name="ot")
    nc.vector.tensor_scalar_mul(ot[:], acc[:, 0:64], rec[:, 0:1])
    nc.vector.tensor_tensor(out=ot[:], in0=ot[:], in1=b2_sb[:], op=A.add)
    nc.sync.dma_start(out=outp[t * 128:(t + 1) * 128, :], in_=ot[:])


# ---------------------------------------------------------------------------
# entry point
# ---------------------------------------------------------------------------

def make_in_maps(T, common, per_core):
    in_maps = []
    for c in range(NCORES):
        m = {
            "xT": common["xT"], "W1ext": common["W1ext"], "W2f": common["W2f"],
            "b1rep": common["b1rep"], "b2rep": common["b2rep"],
            "as2rep": common["as2rep"], "ad2rep": common["ad2rep"],
            "iotarep": common["iotarep"], "pcol": common["pcol"],
        }
        m.update(per_core[c])
        in_maps.append(m)
    return in_maps


def kernel(**inputs):
    T, common, per_core = host_prep(inputs)
    nc = build_nc(T)
    in_maps = make_in_maps(T, common, per_core)
    res = run_bass_kernel_spmd(nc, in_maps, core_ids=list(range(NCORES)))
    allrows = np.concatenate([res.results[c]["out"] for c in range(NCORES)],
                             axis=0)
    return allrows[common["slotrow"]].astype(np.float32)
